# revision 1
# baseline (speedup 1.0000x reference)
"""GCN layer (h@W scaled by norm, gather/scatter-sum over edges, norm+bias+relu)
as a distributed Bass kernel on 8 TRN2 NeuronCores.

Strategy (DMA-byte-minimized, latency-pipelined):
  out = relu(norm_dst * ((A @ (norm_src*h)) @ W) + bias)   [linearity of matmul]
  - norm_src is folded into the replicated bf16 h table on the host, so the
    scatter matrix S holds pure integer edge multiplicities - exactly
    representable in fp8e4 (halves the S stream); the aggregation matmul runs
    mixed fp8(lhsT) x bf16(rhs), which TRN2 supports at full rate.
  - dst nodes are packed into 160 blocks of <=128 slots with degree-aware LPT
    balancing (uniform edges/block -> uniform unique-src/block); one small
    "starter" block per core leads the schedule so the PE pipeline fills fast;
    remaining blocks are snake-assigned by tile count so the SPMD per-rank
    schedule (max over cores) stays tight (~313 src tiles/core).
  - Per block, the unique src rows are fetched with two half-block SWDGE
    dma_gathers on alternating queues (finer completion granularity keeps the
    in-order consumer fed; 48KB descriptor ring lets generation run ahead).
    A 16-row dummy gather pays the one-time ~12us SWDGE init early, and ~22
    scratch matmuls keep the PE array busy (HAM warm) until real data lands.
  - TensorE: accumulate S_t.T @ G_t into PSUM -> x [128,512] fp32; DVE scales
    by norm_dst (exact fp32) casting to bf16; PE transposes 128x128 chunks;
    bf16 projection with W; +bias on DVE; Relu+bf16-cast on ScalarE; bf16
    stores (host upcasts to fp32).
"""

import numpy as np
import ml_dtypes

import concourse.bacc as bacc
import concourse.mybir as mybir
import concourse.tile as tile
from concourse._compat import cdiv
from concourse.masks import make_identity

N_CORES = 8
BS = 128  # dst block size == partition count
N_SWDGE_QUEUES = 4
NGBUF = 6  # gather buffer ring depth

F32 = mybir.dt.float32
BF16 = mybir.dt.bfloat16
F8E4 = mybir.dt.float8e4
I16 = mybir.dt.int16


def _pack_blocks(deg, n_blocks, n_small, small_edges):
    """Pack nodes into blocks balancing edge counts (LPT), cap BS slots.

    The first n_small blocks are 'starter' blocks capped at ~small_edges
    edges each (gathered fast, so TensorE starts early and warms up while
    the full-size gathers stream in)."""
    import heapq
    order = np.argsort(-deg, kind="stable")
    assign = np.empty(len(deg), np.int64)
    # seed starter blocks from the lowest-degree nodes; they must absorb at
    # least the slot-capacity deficit of the remaining blocks
    need = max(0, len(deg) - (n_blocks - n_small) * BS)
    quota = cdiv(need, n_small) if need else 0
    counts = np.zeros(n_small, np.int64)
    edges = np.zeros(n_small, np.int64)
    pos = len(order) - 1
    for s in range(n_small):
        while counts[s] < BS and (counts[s] < quota
                                  or edges[s] + deg[order[pos]] <= small_edges):
            assign[order[pos]] = s
            counts[s] += 1
            edges[s] += deg[order[pos]]
            pos -= 1
    order = order[:pos + 1]
    heap = [(0, b) for b in range(n_small, n_blocks)]
    heapq.heapify(heap)
    counts = np.zeros(n_blocks, np.int64)
    for node in order:
        spill = []
        while True:
            e, b = heapq.heappop(heap)
            if counts[b] < BS:
                break
            spill.append((e, b))
        assign[node] = b
        counts[b] += 1
        heapq.heappush(heap, (e + int(deg[node]), b))
        for it in spill:
            heapq.heappush(heap, it)
    return assign


def _prepare(h, weight, bias, norm, src, dst):
    """Host-side sharding/preprocessing. Returns (nc, in_maps, meta)."""
    h = np.asarray(h, dtype=np.float32)
    weight = np.asarray(weight, dtype=np.float32)
    bias = np.asarray(bias, dtype=np.float32).reshape(1, -1)
    norm = np.asarray(norm, dtype=np.float32).reshape(-1)
    src = np.asarray(src).astype(np.int64)
    dst = np.asarray(dst).astype(np.int64)

    n_nodes, d_in = h.shape
    d_out = weight.shape[1]
    assert d_in % BS == 0 and d_out % BS == 0

    # norm_src folded into the table; S becomes integer multiplicities.
    h16 = (norm[:, None] * h).astype(ml_dtypes.bfloat16)

    nblk = cdiv(cdiv(n_nodes, N_CORES), BS)      # blocks per core
    n_blocks = N_CORES * nblk
    deg = np.bincount(dst, minlength=n_nodes)
    n_small = N_CORES
    assign = _pack_blocks(deg, n_blocks, n_small, 512)

    # slot of each node within its block
    border = np.lexsort((np.arange(n_nodes), assign))
    slot_of_node = np.empty(n_nodes, np.int64)
    blk_nodes = [[] for _ in range(n_blocks)]
    for node in border:
        b = assign[node]
        slot_of_node[node] = len(blk_nodes[b])
        blk_nodes[b].append(node)

    # group edges by block
    eblk = assign[dst]
    eorder = np.argsort(eblk, kind="stable")
    ecnt = np.bincount(eblk, minlength=n_blocks)
    estart = np.zeros(n_blocks + 1, np.int64)
    np.cumsum(ecnt, out=estart[1:])

    uniq_l, rows_l, slots_l, tiles_b = [], [], [], np.zeros(n_blocks, np.int64)
    for b in range(n_blocks):
        eidx = eorder[estart[b]:estart[b + 1]]
        uniq, inv = np.unique(src[eidx], return_inverse=True)
        uniq_l.append(uniq)
        rows_l.append(inv)                       # stream row per edge
        slots_l.append(slot_of_node[dst[eidx]])  # dst slot per edge
        tiles_b[b] = cdiv(max(len(uniq), 1), BS)

    # ranks 0-3 = starter blocks (small, fast first gathers -> PE warms up
    # on a continuous stream); remaining blocks snake-assigned by tile count
    n_starter_ranks = n_small // N_CORES
    core_blocks = np.empty((N_CORES, nblk), np.int64)
    for r in range(n_starter_ranks):
        core_blocks[:, r] = np.arange(N_CORES) + r * N_CORES
    bo = n_small + np.argsort(-tiles_b[n_small:], kind="stable")
    for r in range(n_starter_ranks, nblk):
        i = r - n_starter_ranks
        row = bo[i * N_CORES:(i + 1) * N_CORES]
        if i % 2:
            row = row[::-1]
        core_blocks[:, r] = row
    t_sched = [int(max(tiles_b[core_blocks[c, r]] for c in range(N_CORES)))
               for r in range(nblk)]
    t_min = [int(min(tiles_b[core_blocks[c, r]] for c in range(N_CORES)))
             for r in range(nblk)]
    t_total = int(sum(t_sched))
    e_pad = t_total * BS

    # padding indices point at row 0 (real data, S=0 there; negative-index
    # skipping hangs the SWDGE ucode when an engine gets zero descriptors)
    src_pack = np.zeros((N_CORES, e_pad), np.int16)
    stab32 = np.zeros((BS, e_pad), np.float32)
    stab = np.zeros((N_CORES, BS, e_pad), ml_dtypes.float8_e4m3)
    ndst = np.zeros((N_CORES, BS, nblk), np.float32)
    node_map = np.full((N_CORES, nblk, BS), -1, np.int64)
    for c in range(N_CORES):
        stab32[:] = 0.0
        off = 0
        for r in range(nblk):
            b = int(core_blocks[c, r])
            uniq = uniq_l[b]
            if len(uniq):
                src_pack[c, off:off + len(uniq)] = uniq.astype(np.int16)
            rows = off + rows_l[b]
            np.add.at(stab32, (rows % BS, (rows // BS) * BS + slots_l[b]), 1.0)
            nodes = blk_nodes[b]
            node_map[c, r, :len(nodes)] = nodes
            ndst[c, :len(nodes), r] = norm[nodes]
            off += t_sched[r] * BS
        stab[c] = stab32.astype(ml_dtypes.float8_e4m3)

    def wrap16(a):  # [e_pad] -> [128, e_pad//16] (16-partition wrap, x8 copies)
        return np.tile(a.reshape(-1, 16).T, (8, 1))

    w16 = weight.astype(ml_dtypes.bfloat16)
    in_maps = []
    for c in range(N_CORES):
        in_maps.append({
            "htab": h16,
            "wmat": w16,
            "bvec": np.tile(bias, (BS, 1)),
            "ndst": ndst[c],
            "gidx": wrap16(src_pack[c]).astype(np.int16),
            "stab": stab[c],
        })

    nc = _build(n_nodes, d_in, d_out, nblk, t_sched, t_min, n_starter_ranks)

    meta = dict(nblk=nblk, node_map=node_map, n_nodes=n_nodes, d_out=d_out)
    return nc, in_maps, meta


def _build(n_nodes, d_in, d_out, nblk, t_sched, t_min, n_start=1):
    """Build the SPMD single-core program (same for all cores)."""
    kin = d_in // BS
    t_total = sum(t_sched)
    e_pad = t_total * BS
    t_max = max(t_sched)

    nc = bacc.Bacc("TRN2", target_bir_lowering=False, debug=False,
                   num_swdge_queues=N_SWDGE_QUEUES,
                   dynamic_dma_scratch_size=49152)
    htab = nc.dram_tensor("htab", [n_nodes, d_in], BF16, kind="ExternalInput")
    wmat = nc.dram_tensor("wmat", [d_in, d_out], BF16, kind="ExternalInput")
    bvec = nc.dram_tensor("bvec", [BS, d_out], F32, kind="ExternalInput")
    ndst = nc.dram_tensor("ndst", [BS, nblk], F32, kind="ExternalInput")
    gidx = nc.dram_tensor("gidx", [128, e_pad // 16], I16, kind="ExternalInput")
    stab = nc.dram_tensor("stab", [BS, e_pad], F8E4, kind="ExternalInput")
    yout = nc.dram_tensor("yout", [nblk * BS, d_out], BF16, kind="ExternalOutput")

    with tile.TileContext(nc) as tc:
        with (
            tc.tile_pool(name="const", bufs=1) as cpool,
            tc.tile_pool(name="gather", bufs=1) as gpool,
            tc.tile_pool(name="sload", bufs=10) as spool,
            tc.tile_pool(name="work", bufs=4) as wpool,
            tc.tile_pool(name="out", bufs=1) as opool,
            tc.tile_pool(name="psx", bufs=3, space="PSUM") as psx,
            tc.tile_pool(name="pst", bufs=2, space="PSUM") as pst,
            tc.tile_pool(name="pso", bufs=2, space="PSUM") as pso,
            tc.tile_pool(name="psw", bufs=1, space="PSUM") as psw,
        ):
            # gather buffers: two tiles per ring slot (half-gathers -> finer
            # completion granularity, matmuls start on the first half)
            HA = 6
            gta, gtb = [], []
            for i in range(NGBUF):
                ga = gpool.tile([128, HA, d_in], BF16, tag=f"ga{i}", name=f"ga{i}")
                gb = gpool.tile([128, t_max - HA, d_in], BF16, tag=f"gb{i}",
                                name=f"gb{i}")
                gta.append(ga)
                gtb.append(gb)
            # tiny dummy gathers: pay SWDGE init latency on each queue before
            # the real pipeline needs it (idx tile memset=0 -> row 0, 16x)
            widx = cpool.tile([128, 1], I16, tag="widx")
            nc.gpsimd.memset(widx[:], 0)
            nc.gpsimd.dma_gather(
                gta[NGBUF - 1][:, 0:1, :], htab[:, :], widx[:, 0:1],
                16, 16, d_in, single_packet=False, queue_num=0,
            )
            # starter blocks' indices load separately so gathers start asap
            s0 = sum(t_sched[0:n_start]) * 8
            idxt0 = cpool.tile([128, s0], I16, tag="idx0")
            nc.sync.dma_start(idxt0[:], gidx[:, 0:s0])
            idxt = cpool.tile([128, e_pad // 16], I16, tag="idx")
            nc.sync.dma_start(idxt[:, s0:], gidx[:, s0:])
            ident = cpool.tile([BS, BS], BF16)
            make_identity(nc, ident[:])
            # PE warm-up: ~50 matmuls on scratch keep the array busy from
            # ~7us until the first gather lands, so HAM reaches K=8/8 before
            # the real stream starts (cold MMs run at half clock)
            scr = cpool.tile([128, d_out], BF16, tag="scr")
            nc.gpsimd.memset(scr[:], 0)
            pw = psw.tile([BS, d_out], F32, tag="pw")
            for _ in range(22):
                nc.tensor.matmul(pw[:], ident[:], scr[:], start=True, stop=True)
            ws = cpool.tile([128, kin, d_out], BF16)
            nc.sync.dma_start(ws[:], wmat[:].rearrange("(k p) n -> p k n", p=128))
            bs_t = cpool.tile([128, d_out], F32)
            nc.sync.dma_start(bs_t[:], bvec[:])
            ns_t = cpool.tile([BS, nblk], F32)
            nc.sync.dma_start(ns_t[:], ndst[:])


            otiles = []
            off = 0  # edge-tile offset
            for j in range(nblk):
                tj = t_sched[j]
                ga, gb = gta[j % NGBUF], gtb[j % NGBUF]
                ha = min(tj, HA)
                q = (2 * j) % N_SWDGE_QUEUES
                qb = (2 * j + 1) % N_SWDGE_QUEUES
                full_idx = idxt0 if j < n_start else idxt
                ioff = off * 8
                nc.gpsimd.dma_gather(
                    ga[:, 0:ha, :], htab[:, :],
                    full_idx[:, ioff:ioff + ha * 8],
                    ha * BS, ha * BS, d_in, single_packet=False, queue_num=q,
                )
                if tj > ha:
                    nc.gpsimd.dma_gather(
                        gb[:, 0:tj - ha, :], htab[:, :],
                        full_idx[:, ioff + ha * 8:ioff + tj * 8],
                        (tj - ha) * BS, (tj - ha) * BS, d_in,
                        single_packet=False, queue_num=qb,
                    )
                st = spool.tile([BS, t_max * BS], F8E4, tag="St")
                nc.scalar.dma_start(st[:, 0:tj * BS],
                                    stab[:, off * BS:(off + tj) * BS])
                px = psx.tile([BS, d_in], F32, tag="px")
                for t in range(tj):
                    gsl = ga[:, t, :] if t < ha else gb[:, t - ha, :]
                    nc.tensor.matmul(px[:], st[:, t * BS:(t + 1) * BS],
                                     gsl, start=(t == 0),
                                     stop=(t == tj - 1))
                off += tj

                # x scaled by norm_dst (fp32->bf16), transpose, project, relu
                xs = wpool.tile([BS, d_in], BF16, tag="xs")
                nc.vector.tensor_scalar(xs[:], px[:], ns_t[:, j:j + 1], None,
                                        mybir.AluOpType.mult)
                xT = wpool.tile([128, kin, BS], BF16, tag="xT")
                for k in range(kin):
                    tp = pst.tile([BS, BS], BF16, tag="tp")
                    nc.tensor.transpose(tp[:], xs[:, k * BS:(k + 1) * BS], ident[:])
                    nc.vector.tensor_copy(xT[:, k, :], tp[:])
                po = pso.tile([BS, d_out], F32, tag="po")
                for k in range(kin):
                    nc.tensor.matmul(po[:], xT[:, k, :], ws[:, k, :],
                                     start=(k == 0), stop=(k == kin - 1))
                pb = wpool.tile([BS, d_out], F32, tag="pb")
                nc.vector.tensor_tensor(pb[:], po[:], bs_t[:],
                                        mybir.AluOpType.add)
                ot = opool.tile([BS, d_out], BF16, tag=f"ot{j}", name=f"ot{j}")
                nc.scalar.activation(ot[:], pb[:],
                                     mybir.ActivationFunctionType.Relu)
                otiles.append(ot)

            # stores: block 16 first, then 0..15 FIFO-blocked behind it on the
            # sync queue -> their DMA fires in the post-gather tail (idle DMA)
            # instead of stealing bandwidth from the saturated gather phase
            late = min(16, nblk - 1)
            order = [late] + list(range(late)) + list(range(late + 1, nblk))
            for j in order:
                nc.sync.dma_start(yout[j * BS:(j + 1) * BS, :], otiles[j][:])

    nc.compile()
    return nc


def _assemble(results, meta):
    n_nodes, d_out = meta["n_nodes"], meta["d_out"]
    nblk = meta["nblk"]
    node_map = meta["node_map"]
    out = np.empty((n_nodes, d_out), np.float32)
    for c in range(N_CORES):
        res = np.asarray(results[c]["yout"]).astype(np.float32)
        nm = node_map[c].reshape(-1)
        valid = nm >= 0
        out[nm[valid]] = res[valid]
    return out


def kernel(h, weight, bias, norm, src, dst):
    from concourse.bass_utils import run_bass_kernel_spmd
    nc, in_maps, meta = _prepare(h, weight, bias, norm, src, dst)
    r = run_bass_kernel_spmd(nc, in_maps, list(range(N_CORES)))
    return _assemble(r.results, meta)

